# revision 1
# baseline (speedup 1.0000x reference)
"""Multi-head attention (B=4, S=2048, C=768, H=8, HD=96) on 8 TRN2 NeuronCores.

Strategy: tensor-parallel by head — one head per core. All TensorEngine
matmuls run bf16 inputs with f32 PSUM accumulation.

  - QKV is computed directly transposed: qT/kT/vT [HD, tok] = W_chunk.T @ xT
    with the weight chunk stationary and 512-token moving operand (N=512),
    avoiding any PE transposes of x or q/k. xT and all weights are host
    pre-transposed bf16.
  - RoPE runs in the transposed layout: the pair-swap is one PE matmul with a
    host-provided [96,96] swap matrix; the sign lives in the sin table
    (s[2i] = -sin[2i]). Tables are host-transposed to [HD, S].
  - v is moved to k-major layout with 2-byte DMA transposes (XBAR), off PE.
  - Attention per (b, q-tile): scores.T [k,q] = kT.T @ qT on PE, exp on ACT
    (scale folded in; no max-subtraction needed: scores ~ N(0,1)), P.T (bf16)
    feeds PV directly: out.T [HD+1, q] = v_aug.T @ P.T, where v is augmented
    with a ones column so row HD accumulates the softmax denominator.
  - Normalization: reciprocal in token-parallel layout, then a stride-0-DMA
    broadcast of the reciprocal row from a DRAM bounce (no PE involved).
  - Token ownership is round-robin per batch (core h owns tokens
    [h*256,(h+1)*256) of every batch); each PAIR of batches shares one small
    AllToAll + one N=512 projection pass, so comm and projection overlap the
    following batches' attention and only the last pair's collective is on
    the critical tail.
  - PSUM pools are disjoint per pipeline phase so Tile's in-order slot
    granting never serializes one phase behind another.
"""

import numpy as np
from contextlib import ExitStack

import concourse.bass as bass
from concourse import bacc
import concourse.tile as tile
from concourse import mybir
from concourse.bass_utils import run_bass_kernel_spmd

B, S, C, H, HD = 4, 2048, 768, 8, 96
T = B * S            # 8192 tokens
NCORES = 8
TSLICE = T // NCORES  # 1024 tokens per core for the projection
BSLICE = S // NCORES  # 256 tokens per (core, batch)
KC = C // 128        # 6 contraction chunks of 128
F32 = mybir.dt.float32
BF16 = mybir.dt.bfloat16


def build_nc():
    nc = bacc.Bacc(None, num_devices=NCORES)

    xT = nc.declare_dram_parameter("xT", [C, T], BF16, isOutput=False)
    wqkvT = nc.declare_dram_parameter("wqkvT", [C, 3 * HD], BF16, isOutput=False)
    wprojT = nc.declare_dram_parameter("wprojT", [C, C], BF16, isOutput=False)
    cosT = nc.declare_dram_parameter("cosT", [HD, S], F32, isOutput=False)
    sT = nc.declare_dram_parameter("sT", [HD, S], F32, isOutput=False)
    biasd = nc.declare_dram_parameter("bias", [128, KC], F32, isOutput=False)
    outd = nc.declare_dram_parameter("out", [C, TSLICE], F32, isOutput=True)

    a2a_in = [nc.dram_tensor(f"a2a_in{p}", [C, 2 * BSLICE], BF16) for p in range(B // 2)]
    a2a_out = [nc.dram_tensor(f"a2a_out{p}", [C, 2 * BSLICE], BF16) for p in range(B // 2)]
    dnb = nc.dram_tensor("dnb", [1, 512], F32)  # denominator-reciprocal bounce

    SCALE = HD ** -0.5
    MULT = mybir.AluOpType.mult
    ADD = mybir.AluOpType.add
    EXP = mybir.ActivationFunctionType.Exp
    IDENT = mybir.ActivationFunctionType.Identity

    with tile.TileContext(nc, num_cores=NCORES) as tc, ExitStack() as ctx:
        const = ctx.enter_context(tc.tile_pool(name="const", bufs=1))
        xtp = ctx.enter_context(tc.tile_pool(name="xtp", bufs=2))
        rawp = ctx.enter_context(tc.tile_pool(name="rawp", bufs=4))
        ropep = ctx.enter_context(tc.tile_pool(name="ropep", bufs=4))
        Pp = ctx.enter_context(tc.tile_pool(name="Pp", bufs=4))
        nrm = ctx.enter_context(tc.tile_pool(name="nrm", bufs=4))
        rcp = ctx.enter_context(tc.tile_pool(name="rcp", bufs=3))
        yp = ctx.enter_context(tc.tile_pool(name="yp", bufs=3))
        agcp = ctx.enter_context(tc.tile_pool(name="agcp", bufs=2))

        # PSUM (8 banks), pools disjoint per phase:
        #   qkv accumulators 3 + scores 2 + PV acc 2 + proj 1
        psqkv = ctx.enter_context(tc.tile_pool(name="psqkv", bufs=3, space="PSUM"))
        pssc = ctx.enter_context(tc.tile_pool(name="pssc", bufs=2, space="PSUM"))
        psacc = ctx.enter_context(tc.tile_pool(name="psacc", bufs=2, space="PSUM"))
        pspy = ctx.enter_context(tc.tile_pool(name="pspy", bufs=1, space="PSUM"))

        # --- constants ---
        wq_sb = const.tile([128, KC, 3 * HD], BF16)
        nc.sync.dma_start(wq_sb, wqkvT.ap().rearrange("(kc p) n -> p kc n", p=128))
        wp_sb = const.tile([128, KC, C], BF16)
        nc.sync.dma_start(wp_sb, wprojT.ap().rearrange("(kc p) n -> p kc n", p=128))
        cosT_sb = const.tile([HD, S], F32)
        nc.sync.dma_start(cosT_sb, cosT.ap())
        sT_sb = const.tile([HD, S], F32)
        nc.sync.dma_start(sT_sb, sT.ap())
        bias_sb = const.tile([128, KC], F32)
        nc.sync.dma_start(bias_sb, biasd.ap())

        # persistent ping/pong per-batch q/k (transposed, channel-padded) and v
        qT = [const.tile([128, S], BF16, name=f"qT{i}") for i in range(2)]
        kT = [const.tile([128, S], BF16, name=f"kT{i}") for i in range(2)]
        vA = [const.tile([128, 16, 128], BF16, name=f"vA{i}") for i in range(2)]
        for i in range(2):
            nc.vector.memset(qT[i][HD:128, :], 0.0)
            nc.vector.memset(kT[i][HD:128, :], 0.0)


        vaug = [const.tile([128, 512], BF16, name=f"vaug{i}") for i in range(2)]
        for i in range(2):
            nc.vector.memset(vaug[i][HD:128, :], 0.0)
            nc.vector.memset(vaug[i][HD:HD + 1, :], 1.0)

        xTv = xT.ap().rearrange("(kc p) t -> p kc t", p=128)  # [128, KC, T]

        def do_proj(p):
            """Projection for this core's 512 tokens of batch pair p."""
            W = 2 * BSLICE
            agc = agcp.tile([128, KC, W], BF16)
            nc.sync.dma_start(
                agc, a2a_out[p].ap().rearrange("(kc p) t -> p kc t", p=128))
            if p == B // 2 - 1:
                # tail pair: qkv banks are idle — interleave 3 accumulators
                for kog in range(2):
                    pys = [psqkv.tile([128, W], F32, tag="qkv", name=f"pyt{i}")
                           for i in range(3)]
                    for kc in range(KC):
                        for i in range(3):
                            ko = kog * 3 + i
                            nc.tensor.matmul(
                                pys[i], wp_sb[:, kc, ko * 128:(ko + 1) * 128],
                                agc[:, kc, :],
                                start=(kc == 0), stop=(kc == KC - 1),
                            )
                    for i in range(3):
                        ko = kog * 3 + i
                        y_sb = yp.tile([128, W], F32)
                        nc.scalar.activation(
                            y_sb, pys[i], IDENT,
                            bias=bias_sb[:, ko:ko + 1], scale=1.0,
                        )
                        nc.sync.dma_start(
                            outd.ap()[ko * 128:(ko + 1) * 128, p * W:(p + 1) * W],
                            y_sb)
            else:
                for ko in range(KC):
                    y_sb = yp.tile([128, W], F32)
                    py = pspy.tile([128, W], F32)
                    for kc in range(KC):
                        nc.tensor.matmul(
                            py, wp_sb[:, kc, ko * 128:(ko + 1) * 128],
                            agc[:, kc, :],
                            start=(kc == 0), stop=(kc == KC - 1),
                        )
                    nc.scalar.activation(
                        y_sb, py, IDENT, bias=bias_sb[:, ko:ko + 1], scale=1.0,
                    )
                    nc.sync.dma_start(
                        outd.ap()[ko * 128:(ko + 1) * 128, p * W:(p + 1) * W],
                        y_sb)

        SWAPMASK = []
        for i in range(16):
            SWAPMASK += [2 * i + 1, 2 * i]

        def qkv_group(b, g):
            q_b, k_b, v_b = qT[b % 2], kT[b % 2], vA[b % 2]
            tok0 = b * S + g * 512
            seq = slice(g * 512, (g + 1) * 512)
            xts = []
            for kc in range(KC):
                xtc = xtp.tile([128, 512], BF16, tag=f"xtc{kc}", name=f"xtc{kc}")
                nc.sync.dma_start(xtc, xTv[:, kc, tok0:tok0 + 512])
                xts.append(xtc)
            # interleaved q/k/v accumulation across three PSUM banks so
            # consecutive matmuls never target the same bank
            ps = [psqkv.tile([HD, 512], F32, tag="qkv", name=f"qkvps{ti}") for ti in range(3)]
            for kc in range(KC):
                for ti in range(3):
                    nc.tensor.matmul(
                        ps[ti], wq_sb[:, kc, ti * HD:(ti + 1) * HD], xts[kc],
                        start=(kc == 0), stop=(kc == KC - 1),
                    )
            for ti, dstT in ((0, q_b), (1, k_b)):
                raw = rawp.tile([HD, 512], BF16, tag="raw")
                nc.vector.tensor_copy(out=raw, in_=ps[ti])
                rot = rawp.tile([HD, 512], BF16, tag="rot")
                nc.vector.stream_shuffle(rot, raw, SWAPMASK)
                t1 = ropep.tile([HD, 512], F32, tag="t1")
                nc.vector.tensor_tensor(t1, raw, cosT_sb[:, seq], MULT)
                t2 = ropep.tile([HD, 512], F32, tag="t2")
                nc.vector.tensor_tensor(t2, rot, sT_sb[:, seq], MULT)
                nc.vector.tensor_tensor(
                    dstT[0:HD, g * 512:(g + 1) * 512], t1, t2, ADD)
            vraw = vaug[g % 2]
            nc.vector.tensor_copy(out=vraw[0:HD, :], in_=ps[2])
            for c in range(4):
                nc.sync.dma_start(
                    out=v_b[:, 4 * g + c, :],
                    in_=vraw[:, c * 128:(c + 1) * 128],
                    transpose=True,
                )

        def attention_tile(b, qt):
            q_b, k_b, v_b = qT[b % 2], kT[b % 2], vA[b % 2]
            acc = psacc.tile([128, 512], F32)
            for kt in range(16):
                sc = pssc.tile([128, 512], F32)
                nc.tensor.matmul(
                    sc, k_b[:, kt * 128:(kt + 1) * 128],
                    q_b[:, qt * 512:(qt + 1) * 512],
                    start=True, stop=True,
                )
                Pt = Pp.tile([128, 512], BF16)
                nc.scalar.activation(Pt, sc, EXP, scale=SCALE)
                nc.tensor.matmul(
                    acc, v_b[:, kt, :], Pt,
                    start=(kt == 0), stop=(kt == 15),
                )
            # normalize: reciprocal of denominators (row HD of acc), then a
            # stride-0 DMA broadcast of the reciprocal row from DRAM
            dnrow = rcp.tile([1, 512], F32, tag="dnrow")
            nc.vector.tensor_copy(out=dnrow, in_=acc[HD:HD + 1, :])
            dn = rcp.tile([128, 4], F32, tag="dn")
            nc.sync.dma_start(dn, dnrow)
            rc = rcp.tile([128, 4], F32, tag="rc")
            nc.vector.reciprocal(rc, dn)
            nc.sync.dma_start(dnb.ap(), rc)
            bcast = nrm.tile([HD, 512], F32, tag="bcast")
            dnb_ap = dnb.ap()
            bcast_src = bass.AP(
                tensor=dnb_ap.tensor, offset=dnb_ap.offset,
                ap=[[0, HD]] + list(dnb_ap.ap)[1:],
            )
            nc.sync.dma_start(bcast, bcast_src)
            onorm = nrm.tile([HD, 512], BF16, tag="onorm")
            nc.vector.tensor_tensor(onorm, acc[0:HD, :], bcast, MULT)
            for half in range(2):
                j = 2 * qt + half
                co = (b % 2) * BSLICE
                nc.sync.dma_start(
                    a2a_in[b // 2].ap()[j * HD:(j + 1) * HD, co:co + BSLICE],
                    onorm[:, half * 256:(half + 1) * 256])

        # prologue: batch 0 qkv
        for g in range(4):
            qkv_group(0, g)
        for b in range(B):
            # attention(b) interleaved with qkv(b+1) in program order so the
            # scheduler spreads the next batch's prep across this batch
            for qt in range(4):
                attention_tile(b, qt)
                if b + 1 < B:
                    qkv_group(b + 1, qt)
            if b % 2 == 1:
                nc.gpsimd.collective_compute(
                    "AllToAll", mybir.AluOpType.bypass,
                    replica_groups=[list(range(NCORES))],
                    ins=[a2a_in[b // 2].ap().opt()],
                    outs=[a2a_out[b // 2].ap().opt()],
                )
                do_proj(b // 2)

    nc.compile()
    return nc


_NC_CACHE = None


def _get_nc():
    global _NC_CACHE
    if _NC_CACHE is None:
        _NC_CACHE = build_nc()
    return _NC_CACHE


def make_in_maps(x, cos, sin, Wqkv, Wproj, bproj):
    import ml_dtypes

    bf16 = ml_dtypes.bfloat16
    x = np.asarray(x, np.float32)
    cos = np.asarray(cos, np.float32)
    sin = np.asarray(sin, np.float32)
    Wqkv = np.asarray(Wqkv, np.float32)
    Wproj = np.asarray(Wproj, np.float32)
    bproj = np.asarray(bproj, np.float32)

    xT = np.ascontiguousarray(x.reshape(T, C).T.astype(bf16))  # [C, T] bf16
    wprojT = np.ascontiguousarray(Wproj.T.astype(bf16))        # [C_in, C_out]
    s = sin.copy()
    s[:, 0::2] = -sin[:, 0::2]
    cosT = np.ascontiguousarray(cos.T)                         # [HD, S] f32
    sT = np.ascontiguousarray(s.T)                             # [HD, S] f32
    bias2 = np.ascontiguousarray(bproj.reshape(KC, 128).T)     # [128, KC]

    in_maps = []
    for h in range(NCORES):
        wh = np.concatenate(
            [
                Wqkv[h * HD:(h + 1) * HD],                 # q rows
                Wqkv[C + h * HD:C + (h + 1) * HD],         # k rows
                Wqkv[2 * C + h * HD:2 * C + (h + 1) * HD], # v rows
            ],
            axis=0,
        )                                                  # [3*HD, C]
        wqkvT_h = np.ascontiguousarray(wh.T.astype(bf16))  # [C, 3*HD]
        in_maps.append({
            "xT": xT,
            "wqkvT": wqkvT_h,
            "wprojT": wprojT,
            "cosT": cosT,
            "sT": sT,
            "bias": bias2,
        })
    return in_maps


def assemble_output(results):
    # core h's out [C, 4*256]: columns b*256+i -> global token b*S + h*256 + i
    y = np.empty((T, C), np.float32)
    for h in range(NCORES):
        o = results[h]["out"].T  # [1024, C]
        for b in range(B):
            col = (b // 2) * 2 * BSLICE + (b % 2) * BSLICE
            y[b * S + h * BSLICE:b * S + (h + 1) * BSLICE] = \
                o[col:col + BSLICE]
    return y.reshape(B, S, C)


def kernel(x, cos, sin, Wqkv, Wproj, bproj, _trace=False, **run_kwargs):
    nc = _get_nc()
    in_maps = make_in_maps(x, cos, sin, Wqkv, Wproj, bproj)
    res = run_bass_kernel_spmd(
        nc, in_maps, core_ids=list(range(NCORES)), trace=_trace, **run_kwargs
    )
    out = assemble_output(res.results)
    kernel.last_results = res
    return out


if __name__ == "__main__":
    nc = build_nc()
    print("built OK, instructions:", len(nc.inst_map))

